# revision 27
# baseline (speedup 1.0000x reference)
"""MultiHeadAttn1D Trainium2 Bass kernel (dual-engine exp + PE row tiling).

Problem: x (4, 256, 2048) fp32; Wq/Wk (512, 256); Wv (512, 256).
  q = Wq @ x[n]; k = Wk @ x[n]; v = Wv @ x[n]  (per batch n)
  per head h (8 heads, dk=dv=64):
    scores[tk, tq] = sum_d k[d,tk] q[d,tq] / 8; attn = softmax over tk
    out[d, tq] = sum_tk attn[tk,tq] v[d,tk]

Sharding: 8 cores = 4 batch x 2 head-groups. Core c handles n = c//2 and
heads 4*(c%2) .. 4*(c%2)+3 (256 rows of each W). Pure SPMD, no collectives.

Per-core design (matmuls bf16 operands, fp32 PSUM accumulate):
  - Head-PAIR layout: the 4 local heads form 2 pairs; a pair's q/k rows sit
    at partitions 0-63 (even head) / 64-127 (odd head) of one (128, T)
    projection tile. The two K=64 score matmuls of a step (one per head)
    land on disjoint 64-row PE row-groups (tile_position auto-derives from
    the base partitions) and different PSUM banks, so they execute
    CONCURRENTLY in the PE array - measured ~84ns/matmul in a clean stream
    vs 300ns serial; disabling the row split costs +30us on the full
    kernel. No weight duplication needed (v1 doubled q/k projection work).
  - exp runs on TWO engines. ScalarE activation takes 12 of 16 tk-tiles
    per unit at (172+FD)/1.2 ns from PSUM (997ns @ FD=1024); a custom DVE
    op EXP16_ANT (registered additively into dve_ops at import) takes
    steps {5,8,11,14}, reading PSUM directly at 1 elem/cycle/lane. The op
    computes ((c0*s+c1)^2+c2)^16 ~= exp(s/8) via 4 inline squarings
    (8-stage ALU chain; scores satisfy |s| < ~5, fit window |s|<=12; rel
    err ~7e-4 fp32, bf16 cast dominates). A pure mul/add chain is safe
    from PSUM - the bitwise-NOT reciprocal op is the one that misbehaves.
    Both engines write bf16 E tiles; softmax mixes them freely per row.
  - Unit = (head-pair, tq-quarter of 512): 16 steps, one tk-tile each:
    2 concurrent score matmuls -> one FD=1024 exp (both heads) -> TRANSPOSED
    attn@V, lagged 3 steps: lhsT = E chunk (tk, tq128), rhs = vt (tk,
    d|ones), so each matmul streams only N=65 (~4x cheaper than the
    (65, 512) orientation) and the accumulator is [tq_part, chunk, d|sum]
    with sum(exp) landing PER PARTITION in column 64. PSUM pending-zero is
    BANK-granular: exactly one start=True per acc bank (j=0, c=0) - a
    start per chunk re-marks the bank and drops sibling j=0 contributions.
    fp8 attn@V (DoubleRow) was measured 2.3x faster still but fails the
    accuracy budget (~5-7% output error from e4m3 E/v quantization).
  - PSUM: 3 score slots (128,1024)f32 = 6 banks + 2 transposed accs
    (128,4,128)f32 = 1 bank each = 8 exactly.
  - Epilogue per unit per head: one small evac copy (DVE, 396ns), then
    GPSIMD normalize_recip divides by the per-partition sumexp and emits
    bf16, xbar dma_start_transpose flips (tq,128x(c,d)) blocks to output
    orientation, and plain DMAs store bf16 to DRAM (kernel() upcasts to
    fp32 on host; bf16 out costs ~3e-4 extra error). This removes the
    sum-row copy, reciprocal, broadcast, and multiply from the DVE/GPSIMD
    critical path entirely. The last `lag` attn@V steps + the epilogue are
    deferred into the next unit's first interleave slots.
  - Projection pieces / vT pairs / deferred tails interleave one per step
    into the score stream. Scheduler pacing via tile_wait_until was tried
    (pace knob) and measured neutral-to-negative; disabled by default.
"""

import numpy as np
import ml_dtypes

# Problem constants (hardcoded per contract; kernel.py must be self-contained)
N_BATCH = 4
C_IN = 256
T = 2048
C_OUT = 512
H = 8
DK = 64
N_CORES = 8
H_LOC = 4            # heads per core
ROWS = 256           # W rows per core (H_LOC * DK)
TK_TILES = 16        # T / 128
TQ_U = 512           # tq per unit (quarter of T)
N_PAIRS = 2          # head pairs per core
N_Q = 4              # tq quarters

# exp(s/8) ~= ((C0*s + C1)^2 + C2)^16, fit on |s|<=12 (see docstring)
EXP_C0 = 0.005531007068092956
EXP_C1 = 0.7070977101176488
EXP_C2 = 0.5000073251988786

# DVE takes these tk-tile exps in each unit (rest on ScalarE)
DVE_STEPS = (2, 4, 9, 11, 13, 15)

_PROGRAMS = {}


def _register_exp_op():
    """Register EXP16_ANT as a custom DVE op (idempotent, additive)."""
    from concourse import dve_ops
    from concourse.dve_spec import C0, C1, C2, Spec, Src0, lower
    from concourse.dve_table_gen import dve_ver_for
    from concourse.dve_uop import DveOpSpec

    name = "EXP16_ANT"
    for op in dve_ops.OPS:
        if op.name == name:
            return op

    def ref(in0, in1, s0, s1, imm2):
        u = (np.float32(s0) * in0.astype(np.float32) + np.float32(s1)).astype(
            np.float32
        )
        q = (u * u + np.float32(imm2)).astype(np.float32)
        q2 = (q * q).astype(np.float32)
        q4 = (q2 * q2).astype(np.float32)
        q8 = (q4 * q4).astype(np.float32)
        return (q8 * q8).astype(np.float32)

    u = Src0 * C0 + C1
    q = u * u + C2
    q2 = q * q
    q4 = q2 * q2
    q8 = q4 * q4
    spec = Spec(body=q8 * q8, reference=ref)

    ver = dve_ver_for("TRN2")
    row = dve_ops._CUSTOM_DVE_ROW_BASE + len(dve_ops.OPS)
    tmp = DveOpSpec(name=name, opcode=row, uops=lower(spec, ver=ver), rd1_en=False)
    op = dve_ops.DveOp(name, spec, subdim=False, uops_sha={ver: tmp.sha(ver)})
    dve_ops.OPS.append(op)
    dve_ops.CUSTOM_DVE_SPECS[name] = spec
    dve_ops._SUB_OPCODE_FOR_NAME[name] = row
    return op


def _build_program(passes=1, loop_n=None, dve_steps=DVE_STEPS, lag=3,
                   mult_gpsimd=True, e_bufs=12, pace=0, base=0.004,
                   sc_base0=False):
    import concourse.bass as bass  # noqa: F401
    import concourse.tile as tile
    from concourse import bacc, mybir

    BF16 = mybir.dt.bfloat16
    FP32 = mybir.dt.float32
    EXP = mybir.ActivationFunctionType.Exp
    EXPOP = _register_exp_op()

    nc = bacc.Bacc(
        "TRN2",
        target_bir_lowering=False,
        debug=False,
        num_devices=N_CORES,
    )

    xb_d = nc.dram_tensor("xb", [C_IN, T], BF16, kind="ExternalInput").ap()
    wqt_d = nc.dram_tensor("wqt", [C_IN, ROWS], BF16, kind="ExternalInput").ap()
    wkt_d = nc.dram_tensor("wkt", [C_IN, ROWS], BF16, kind="ExternalInput").ap()
    wvt_d = nc.dram_tensor("wvt", [C_IN, ROWS], BF16, kind="ExternalInput").ap()
    out_d = nc.dram_tensor("out", [ROWS, T], BF16, kind="ExternalOutput").ap()

    with tile.TileContext(nc) as tc:
        from contextlib import ExitStack

        with ExitStack() as ctx:
            singles = ctx.enter_context(tc.tile_pool(name="singles", bufs=1))
            psS = ctx.enter_context(tc.tile_pool(name="psS", bufs=3, space="PSUM"))
            psA = ctx.enter_context(tc.tile_pool(name="psA", bufs=2, space="PSUM"))
            eP = ctx.enter_context(tc.tile_pool(name="eP", bufs=e_bufs))
            small = ctx.enter_context(tc.tile_pool(name="small", bufs=6))
            outP = ctx.enter_context(tc.tile_pool(name="outP", bufs=6))

            # ---- persistent SBUF tensors ----
            xb_sb = singles.tile([128, 2, T], BF16, tag="xb", name="xb_sb")
            wqt_sb = singles.tile([128, 2, ROWS], BF16, tag="wqt", name="wqt_sb")
            wkt_sb = singles.tile([128, 2, ROWS], BF16, tag="wkt", name="wkt_sb")
            wvt_sb = singles.tile([128, 2, ROWS], BF16, tag="wvt", name="wvt_sb")

            def chunked(dram_ap, cols, c0=0, c1=None):
                """(256, F) dram AP -> (128, 2, c1-c0) view, chunk-major free."""
                c1 = cols if c1 is None else c1
                import concourse.bass as bass_mod

                return bass_mod.AP(
                    tensor=dram_ap.tensor,
                    offset=dram_ap.offset + c0,
                    ap=[[cols, 128], [128 * cols, 2], [1, c1 - c0]],
                )

            nc.sync.dma_start(out=wkt_sb, in_=chunked(wkt_d, ROWS))
            nc.scalar.dma_start(
                out=xb_sb[:, :, 0:TQ_U], in_=chunked(xb_d, T, 0, TQ_U)
            )
            nc.sync.dma_start(out=wqt_sb, in_=chunked(wqt_d, ROWS))
            nc.scalar.dma_start(
                out=xb_sb[:, :, TQ_U : 2 * TQ_U], in_=chunked(xb_d, T, TQ_U, 2 * TQ_U)
            )
            nc.sync.dma_start(out=wvt_sb, in_=chunked(wvt_d, ROWS))
            nc.scalar.dma_start(
                out=xb_sb[:, :, 2 * TQ_U : T], in_=chunked(xb_d, T, 2 * TQ_U, T)
            )

            # q/k per pair: (128, T) bf16; head pair p covers heads 2p, 2p+1
            q_sb = [
                singles.tile([128, T], BF16, tag=f"q{p}", name=f"q{p}")
                for p in range(N_PAIRS)
            ]
            k_sb = [
                singles.tile([128, T], BF16, tag=f"k{p}", name=f"k{p}")
                for p in range(N_PAIRS)
            ]
            # per tk-tile, per head: [vT | ones] (65 columns, ones last)
            vt_aug = singles.tile([128, TK_TILES, H_LOC, DK + 1], BF16, tag="vt")

            def emit_proj_piece(p, u, wt_sb, dst):
                """One 512-col piece (pair p, quarter u) of a q/k projection."""
                ps = psS.tile([128, TQ_U], FP32, tag="S", name="projps")
                col = TQ_U * u
                for c in range(2):
                    nc.tensor.matmul(
                        ps,
                        lhsT=wt_sb[:, c, 128 * p : 128 * (p + 1)],
                        rhs=xb_sb[:, c, col : col + TQ_U],
                        start=(c == 0),
                        stop=(c == 1),
                    )
                nc.vector.tensor_copy(dst[:, col : col + TQ_U], ps)

            def emit_vt_pair(i):
                """vT for tk-tiles i, i+1 computed into one pool tile."""
                ps = psS.tile([128, 2, H_LOC, DK], FP32, tag="S", name="vtps")
                for t in range(2):
                    for c in range(2):
                        nc.tensor.matmul(
                            ps[:, t],
                            lhsT=xb_sb[:, c, 128 * (i + t) : 128 * (i + t + 1)],
                            rhs=wvt_sb[:, c],
                            start=(c == 0),
                            stop=(c == 1),
                        )
                nc.vector.tensor_copy(vt_aug[:, i : i + 2, :, 0:DK], ps)

            def emit_unit(p, u, interleave=(), defer_tail=True,
                          greedy_items=False, u_idx=0):
                """One (pair, quarter) unit: 16 steps in 4 groups of 4.
                Scores+exp run solid within a group (pure 64x128 PE mode);
                at each group boundary the previous group's attn@V matmuls
                flush as one 128x128-mode block, then up to two `interleave`
                items run. The last group's flush + epilogue are returned as
                closures for the next unit (defer_tail)."""
                interleave = list(interleave)
                # transposed attn@V: acc[h] is [tq_part, chunk, d|sumexp];
                # lhsT = E chunk (tk, tq128), rhs = vt (tk, d|ones): N=65
                # streamed -> ~4x cheaper per element than the (65, 512)
                # orientation, and sumexp lands PER PARTITION (col 64).
                # chunk stride padded to 128 f32 (512B) so each region's
                # start=True zeroing cannot clobber sibling chunks in the bank
                accs = [
                    psA.tile([128, 4, 128], FP32, tag="acc", name=f"acc{h}")
                    for h in range(2)
                ]
                e_tiles = [None] * TK_TILES

                def emit_mm2(j):
                    for h in range(2):
                        for c in range(4):
                            nc.tensor.matmul(
                                accs[h][:, c, 0 : DK + 1],
                                lhsT=e_tiles[j][
                                    :,
                                    TQ_U * h + 128 * c : TQ_U * h + 128 * (c + 1),
                                ],
                                rhs=vt_aug[:, j, 2 * p + h, :],
                                start=(j == 0 and c == 0),
                                stop=(j == TK_TILES - 1 and c == 3),
                            )

                def pop_items(n):
                    while n > 0 and interleave:
                        nxt = interleave.pop(0)
                        n -= 1
                        if nxt is not None:
                            nxt()

                for j in range(TK_TILES):
                  with tc.tile_wait_until(base + pace * (TK_TILES * u_idx + j),
                                          enable=pace > 0):
                    s_tile = psS.tile([128, 2 * TQ_U], FP32, tag="S", name="s")
                    for h in range(2):
                        hb = 0 if sc_base0 else h
                        nc.tensor.matmul(
                            s_tile[:, TQ_U * h : TQ_U * (h + 1)],
                            lhsT=k_sb[p][
                                64 * hb : 64 * hb + 64, 128 * j : 128 * (j + 1)
                            ],
                            rhs=q_sb[p][
                                64 * hb : 64 * hb + 64, TQ_U * u : TQ_U * (u + 1)
                            ],
                            start=True,
                            stop=True,
                        )
                    e = eP.tile([128, 2 * TQ_U], BF16, tag="E", name="e")
                    if j in dve_steps:
                        nc.vector._custom_dve(
                            EXPOP, out=e, in0=s_tile,
                            s0=EXP_C0, s1=EXP_C1, imm2=EXP_C2,
                        )
                    else:
                        nc.scalar.activation(e, s_tile, EXP, scale=0.125)
                    e_tiles[j] = e
                    if j >= lag:
                        emit_mm2(j - lag)
                    pop_items(1)

                def emit_epilogue():
                    for h in range(2):
                        hl = 2 * p + h
                        # evacuate transposed acc (1 bank) -> SBUF
                        av = outP.tile([128, 4, DK + 1], FP32, tag="av", name="av")
                        nc.vector.tensor_copy(av, accs[h][:, :, 0 : DK + 1])
                        # per-partition softmax divide on GPSIMD, bf16 out
                        ot = outP.tile([128, 4, DK], BF16, tag="ot", name="ot")
                        for c in range(4):
                            nc.gpsimd.normalize_recip(
                                ot[:, c, :],
                                av[:, c, 0:DK],
                                av[:, c, DK : DK + 1],
                            )
                        # xbar transpose (tq128, 2x64) -> ((c,d)128, tq128)
                        for b in range(2):
                            tr = small.tile([128, 128], BF16, tag="tr", name="tr")
                            nc.sync.dma_start_transpose(
                                out=tr, in_=ot[:, 2 * b : 2 * b + 2, :]
                            )
                            for cc in range(2):
                                col = TQ_U * u + 128 * (2 * b + cc)
                                nc.sync.dma_start(
                                    out=out_d[
                                        DK * hl : DK * (hl + 1), col : col + 128
                                    ],
                                    in_=tr[64 * cc : 64 * cc + 64, :],
                                )

                while interleave:
                    nxt = interleave.pop(0)
                    if nxt is not None:
                        nxt()

                def tail_a():
                    for j in range(TK_TILES - lag, TK_TILES):
                        emit_mm2(j)

                def tail_b():
                    emit_epilogue()

                if defer_tail:
                    return [tail_a, tail_b]
                tail_a()
                tail_b()
                return []

            # ---- emission order ----
            from functools import partial

            def qp(p, u):
                return partial(emit_proj_piece, p, u, wqt_sb, q_sb[p])

            def kp(p, u):
                return partial(emit_proj_piece, p, u, wkt_sb, k_sb[p])

            def spread(items, lead=1):
                out = [None] * lead
                for it in items:
                    out.extend([it, None])
                return out

            def emit_pass():
                nc.gpsimd.memset(vt_aug, 1.0)
                # unblock the first unit: k pair0 (all quarters), q pair0 q0
                for u in range(N_Q):
                    emit_proj_piece(0, u, wkt_sb, k_sb[0])
                emit_proj_piece(0, 0, wqt_sb, q_sb[0])
                vt = [partial(emit_vt_pair, 2 * i) for i in range(8)]
                # Item slots are popped one per step AFTER that step's exp is
                # emitted. DVE-queue discipline: the deferred epilogue enters
                # at steps 0-1 (prev tail); proj/vT copies go in the DVE idle
                # windows BETWEEN its exp steps (5, 8, 11, 14), never before
                # one, so DVE exps stay early and the score-slot rotation
                # (scores j+1 gate on exp j-2) never stalls ScalarE.
                il = {
                    (0, 0): [vt[0], vt[1], vt[2], vt[3], vt[4], None,
                             vt[5], qp(0, 1), None, vt[6], qp(0, 2), None,
                             vt[7], qp(0, 3)],
                    # prev_tail occupies steps 0-1; il[i] lands at step 2+i
                    (0, 1): [None, None, None, None, kp(1, 0), None, None,
                             kp(1, 1), None, None, kp(1, 2), None, None,
                             kp(1, 3)],
                    (0, 2): [None, None, None, None, qp(1, 0), None, None,
                             qp(1, 1), None, None, qp(1, 2), None, None,
                             qp(1, 3)],
                    (0, 3): [],
                    (1, 0): [],
                    (1, 1): [],
                    (1, 2): [],
                    (1, 3): [],
                }
                prev_tail = []
                order = [(p, u) for p in range(N_PAIRS) for u in range(N_Q)]
                for idx, (p, u) in enumerate(order):
                    items = prev_tail + list(il[(p, u)])
                    last = idx == len(order) - 1
                    prev_tail = emit_unit(
                        p, u, items, defer_tail=not last,
                        greedy_items=(idx == 0), u_idx=idx,
                    )

            if loop_n is not None:
                with tc.For_i(0, loop_n, 1):
                    emit_pass()
            else:
                for _ in range(passes):
                    emit_pass()

    nc.compile()
    return nc


def _get_program(passes=1, loop_n=None, **kw):
    key = (passes, loop_n, tuple(sorted(kw.items())))
    if key not in _PROGRAMS:
        _PROGRAMS[key] = _build_program(passes, loop_n, **kw)
    return _PROGRAMS[key]


def _make_in_maps(inputs):
    x = np.asarray(inputs["x"])
    Wq = np.asarray(inputs["Wq"])
    Wk = np.asarray(inputs["Wk"])
    Wv = np.asarray(inputs["Wv"])
    xb = [np.ascontiguousarray(x[n]).astype(ml_dtypes.bfloat16) for n in range(N_BATCH)]
    rows = [slice(ROWS * g, ROWS * (g + 1)) for g in range(2)]

    def wt(w, r):
        return np.ascontiguousarray(w[r].T).astype(ml_dtypes.bfloat16)

    wqt = [wt(Wq, r) for r in rows]
    wkt = [wt(Wk, r) for r in rows]
    wvt = [wt(Wv, r) for r in rows]
    return [
        {"xb": xb[c // 2], "wqt": wqt[c % 2], "wkt": wkt[c % 2], "wvt": wvt[c % 2]}
        for c in range(N_CORES)
    ]


_CALLABLE = None


def _get_callable():
    """Build the sharded PJRT callable once; repeated kernel() calls reuse it."""
    global _CALLABLE
    if _CALLABLE is not None:
        return _CALLABLE
    import jax
    from jax.sharding import Mesh, PartitionSpec

    from jax.experimental.shard_map import shard_map
    import concourse.bass2jax as b2j
    from concourse import mybir

    nc = _get_program()
    b2j.install_neuronx_cc_hook()
    partition_name = nc.partition_id_tensor.name if nc.partition_id_tensor else None
    in_names, out_names, out_avals, zero_outs = [], [], [], []
    for alloc in nc.m.functions[0].allocations:
        if not isinstance(alloc, mybir.MemoryLocationSet):
            continue
        name = alloc.memorylocations[0].name
        if alloc.kind == "ExternalInput":
            if name != partition_name:
                in_names.append(name)
        elif alloc.kind == "ExternalOutput":
            shape = tuple(alloc.tensor_shape)
            dtype = mybir.dt.np(alloc.dtype)
            out_names.append(name)
            out_avals.append(jax.core.ShapedArray(shape, dtype))
            zero_outs.append(np.zeros(shape, dtype))
    n_params = len(in_names)
    all_in_names = list(in_names) + list(out_names)
    if partition_name is not None:
        all_in_names.append(partition_name)

    def _body(*args):
        operands = list(args)
        if partition_name is not None:
            operands.append(b2j.partition_id_tensor())
        outs = b2j._bass_exec_p.bind(
            *operands,
            out_avals=tuple(out_avals),
            in_names=tuple(all_in_names),
            out_names=tuple(out_names),
            lowering_input_output_aliases=(),
            sim_require_finite=True,
            sim_require_nnan=True,
            nc=nc,
        )
        return tuple(outs)

    devices = jax.devices()[:N_CORES]
    mesh = Mesh(np.asarray(devices), ("core",))
    in_specs = (PartitionSpec("core"),) * (n_params + len(out_names))
    out_specs = (PartitionSpec("core"),) * len(out_names)
    fn = jax.jit(
        shard_map(
            _body, mesh=mesh, in_specs=in_specs, out_specs=out_specs,
            check_rep=False,
        ),
        keep_unused=True,
    )
    concat_zeros = [
        np.zeros((N_CORES * z.shape[0], *z.shape[1:]), z.dtype) for z in zero_outs
    ]
    _CALLABLE = (fn, in_names, out_names, out_avals, concat_zeros)
    return _CALLABLE


def kernel(x, Wq, Wk, Wv):
    fn, in_names, out_names, out_avals, concat_zeros = _get_callable()
    in_maps = _make_in_maps({"x": x, "Wq": Wq, "Wk": Wk, "Wv": Wv})
    concat_in = [
        np.concatenate([in_maps[c][nm] for c in range(N_CORES)], axis=0)
        for nm in in_names
    ]
    out_arrs = fn(*concat_in, *concat_zeros)
    oi = out_names.index("out")
    res = np.asarray(out_arrs[oi]).reshape(N_CORES, *out_avals[oi].shape)

    out = np.empty((N_BATCH, C_OUT, T), np.float32)
    for c in range(N_CORES):
        n = c // 2
        g = c % 2
        out[n, ROWS * g : ROWS * (g + 1), :] = res[c]
    return out


if __name__ == "__main__":
    xs = np.random.randn(N_BATCH, C_IN, T).astype(np.float32)
    wq = (np.random.randn(C_OUT, C_IN) * 0.02).astype(np.float32)
    wk = (np.random.randn(C_OUT, C_IN) * 0.02).astype(np.float32)
    wv = (np.random.randn(C_OUT, C_IN) * 0.02).astype(np.float32)
    o = kernel(xs, wq, wk, wv)
    print("out", o.shape, o.dtype, np.abs(o).max())


# revision 28
# speedup vs baseline: 1.0583x; 1.0583x over previous
"""MultiHeadAttn1D Trainium2 Bass kernel (dual-engine exp + PE row tiling).

Problem: x (4, 256, 2048) fp32; Wq/Wk (512, 256); Wv (512, 256).
  q = Wq @ x[n]; k = Wk @ x[n]; v = Wv @ x[n]  (per batch n)
  per head h (8 heads, dk=dv=64):
    scores[tk, tq] = sum_d k[d,tk] q[d,tq] / 8; attn = softmax over tk
    out[d, tq] = sum_tk attn[tk,tq] v[d,tk]

Sharding: 8 cores = 4 batch x 2 head-groups. Core c handles n = c//2 and
heads 4*(c%2) .. 4*(c%2)+3 (256 rows of each W). Pure SPMD, no collectives.

Per-core design (matmuls bf16 operands, fp32 PSUM accumulate):
  - Head-PAIR layout: the 4 local heads form 2 pairs; a pair's q/k rows sit
    at partitions 0-63 (even head) / 64-127 (odd head) of one (128, T)
    projection tile. The two K=64 score matmuls of a step (one per head)
    land on disjoint 64-row PE row-groups (tile_position auto-derives from
    the base partitions) and different PSUM banks, so they execute
    CONCURRENTLY in the PE array - measured ~84ns/matmul in a clean stream
    vs 300ns serial; disabling the row split costs +30us on the full
    kernel. No weight duplication needed (v1 doubled q/k projection work).
  - exp runs on TWO engines. ScalarE activation takes 12 of 16 tk-tiles
    per unit at (172+FD)/1.2 ns from PSUM (997ns @ FD=1024); a custom DVE
    op EXP16_ANT (registered additively into dve_ops at import) takes
    steps {5,8,11,14}, reading PSUM directly at 1 elem/cycle/lane. The op
    computes ((c0*s+c1)^2+c2)^16 ~= exp(s/8) via 4 inline squarings
    (8-stage ALU chain; scores satisfy |s| < ~5, fit window |s|<=12; rel
    err ~7e-4 fp32, bf16 cast dominates). A pure mul/add chain is safe
    from PSUM - the bitwise-NOT reciprocal op is the one that misbehaves.
    Both engines write bf16 E tiles; softmax mixes them freely per row.
  - Unit = (head-pair, tq-quarter of 512): 16 steps, one tk-tile each:
    2 concurrent score matmuls -> one FD=1024 exp (both heads) -> TRANSPOSED
    attn@V, lagged 3 steps: lhsT = E chunk (tk, tq128), rhs = vt (tk,
    d|ones), so each matmul streams only N=65 (~4x cheaper than the
    (65, 512) orientation) and the accumulator is [tq_part, chunk, d|sum]
    with sum(exp) landing PER PARTITION in column 64. PSUM pending-zero is
    BANK-granular: exactly one start=True per acc bank (j=0, c=0) - a
    start per chunk re-marks the bank and drops sibling j=0 contributions.
    fp8 attn@V (DoubleRow) was measured 2.3x faster still but fails the
    accuracy budget (~5-7% output error from e4m3 E/v quantization).
  - PSUM: 3 score slots (128,1024)f32 = 6 banks + 2 transposed accs
    (128,4,128)f32 = 1 bank each = 8 exactly.
  - Epilogue per unit per head: one small evac copy (DVE, 396ns), then
    GPSIMD normalize_recip divides by the per-partition sumexp and emits
    bf16, xbar dma_start_transpose flips (tq,128x(c,d)) blocks to output
    orientation, and plain DMAs store bf16 to DRAM (kernel() upcasts to
    fp32 on host; bf16 out costs ~3e-4 extra error). This removes the
    sum-row copy, reciprocal, broadcast, and multiply from the DVE/GPSIMD
    critical path entirely. The last `lag` attn@V steps + the epilogue are
    deferred into the next unit's first interleave slots.
  - Projection pieces / vT pairs / deferred tails interleave one per step
    into the score stream. Scheduler pacing via tile_wait_until was tried
    (pace knob) and measured neutral-to-negative; disabled by default.
"""

import numpy as np
import ml_dtypes

# Problem constants (hardcoded per contract; kernel.py must be self-contained)
N_BATCH = 4
C_IN = 256
T = 2048
C_OUT = 512
H = 8
DK = 64
N_CORES = 8
H_LOC = 4            # heads per core
ROWS = 256           # W rows per core (H_LOC * DK)
TK_TILES = 16        # T / 128
TQ_U = 512           # tq per unit (quarter of T)
N_PAIRS = 2          # head pairs per core
N_Q = 4              # tq quarters

# exp(s/8) ~= ((C0*s + C1)^2 + C2)^16, fit on |s|<=12 (see docstring)
EXP_C0 = 0.005531007068092956
EXP_C1 = 0.7070977101176488
EXP_C2 = 0.5000073251988786

# DVE takes these tk-tile exps in each unit (rest on ScalarE). Placement
# rule (worth ~17us): no two steps may differ by 3 = the score-slot count,
# else scores j+3 gates on DVE exp j and the DVE exps chain serially into
# the score critical path, starving ScalarE ((5,8,11,14) measured 131.7us,
# (3,6,9,12,15) 154us, this set 114.4us). Step 15 on DVE also loses (the
# deferred tail attn@V waits on it at the next unit's start): x=6 with
# (2,4,9,11,13,15) measured 139.6us.
DVE_STEPS = (2, 4, 9, 11, 13)

_PROGRAMS = {}


def _register_exp_op():
    """Register EXP16_ANT as a custom DVE op (idempotent, additive)."""
    from concourse import dve_ops
    from concourse.dve_spec import C0, C1, C2, Spec, Src0, lower
    from concourse.dve_table_gen import dve_ver_for
    from concourse.dve_uop import DveOpSpec

    name = "EXP16_ANT"
    for op in dve_ops.OPS:
        if op.name == name:
            return op

    def ref(in0, in1, s0, s1, imm2):
        u = (np.float32(s0) * in0.astype(np.float32) + np.float32(s1)).astype(
            np.float32
        )
        q = (u * u + np.float32(imm2)).astype(np.float32)
        q2 = (q * q).astype(np.float32)
        q4 = (q2 * q2).astype(np.float32)
        q8 = (q4 * q4).astype(np.float32)
        return (q8 * q8).astype(np.float32)

    u = Src0 * C0 + C1
    q = u * u + C2
    q2 = q * q
    q4 = q2 * q2
    q8 = q4 * q4
    spec = Spec(body=q8 * q8, reference=ref)

    ver = dve_ver_for("TRN2")
    row = dve_ops._CUSTOM_DVE_ROW_BASE + len(dve_ops.OPS)
    tmp = DveOpSpec(name=name, opcode=row, uops=lower(spec, ver=ver), rd1_en=False)
    op = dve_ops.DveOp(name, spec, subdim=False, uops_sha={ver: tmp.sha(ver)})
    dve_ops.OPS.append(op)
    dve_ops.CUSTOM_DVE_SPECS[name] = spec
    dve_ops._SUB_OPCODE_FOR_NAME[name] = row
    return op


def _build_program(passes=1, loop_n=None, dve_steps=DVE_STEPS, lag=3,
                   mult_gpsimd=True, e_bufs=12, pace=0, base=0.004,
                   sc_base0=False):
    import concourse.bass as bass  # noqa: F401
    import concourse.tile as tile
    from concourse import bacc, mybir

    BF16 = mybir.dt.bfloat16
    FP32 = mybir.dt.float32
    EXP = mybir.ActivationFunctionType.Exp
    EXPOP = _register_exp_op()

    nc = bacc.Bacc(
        "TRN2",
        target_bir_lowering=False,
        debug=False,
        num_devices=N_CORES,
    )

    xb_d = nc.dram_tensor("xb", [C_IN, T], BF16, kind="ExternalInput").ap()
    wqt_d = nc.dram_tensor("wqt", [C_IN, ROWS], BF16, kind="ExternalInput").ap()
    wkt_d = nc.dram_tensor("wkt", [C_IN, ROWS], BF16, kind="ExternalInput").ap()
    wvt_d = nc.dram_tensor("wvt", [C_IN, ROWS], BF16, kind="ExternalInput").ap()
    out_d = nc.dram_tensor("out", [ROWS, T], BF16, kind="ExternalOutput").ap()

    with tile.TileContext(nc) as tc:
        from contextlib import ExitStack

        with ExitStack() as ctx:
            singles = ctx.enter_context(tc.tile_pool(name="singles", bufs=1))
            psS = ctx.enter_context(tc.tile_pool(name="psS", bufs=3, space="PSUM"))
            psA = ctx.enter_context(tc.tile_pool(name="psA", bufs=2, space="PSUM"))
            eP = ctx.enter_context(tc.tile_pool(name="eP", bufs=e_bufs))
            small = ctx.enter_context(tc.tile_pool(name="small", bufs=6))
            outP = ctx.enter_context(tc.tile_pool(name="outP", bufs=6))

            # ---- persistent SBUF tensors ----
            xb_sb = singles.tile([128, 2, T], BF16, tag="xb", name="xb_sb")
            wqt_sb = singles.tile([128, 2, ROWS], BF16, tag="wqt", name="wqt_sb")
            wkt_sb = singles.tile([128, 2, ROWS], BF16, tag="wkt", name="wkt_sb")
            wvt_sb = singles.tile([128, 2, ROWS], BF16, tag="wvt", name="wvt_sb")

            def chunked(dram_ap, cols, c0=0, c1=None):
                """(256, F) dram AP -> (128, 2, c1-c0) view, chunk-major free."""
                c1 = cols if c1 is None else c1
                import concourse.bass as bass_mod

                return bass_mod.AP(
                    tensor=dram_ap.tensor,
                    offset=dram_ap.offset + c0,
                    ap=[[cols, 128], [128 * cols, 2], [1, c1 - c0]],
                )

            nc.sync.dma_start(out=wkt_sb, in_=chunked(wkt_d, ROWS))
            nc.scalar.dma_start(
                out=xb_sb[:, :, 0:TQ_U], in_=chunked(xb_d, T, 0, TQ_U)
            )
            nc.sync.dma_start(out=wqt_sb, in_=chunked(wqt_d, ROWS))
            nc.scalar.dma_start(
                out=xb_sb[:, :, TQ_U : 2 * TQ_U], in_=chunked(xb_d, T, TQ_U, 2 * TQ_U)
            )
            nc.sync.dma_start(out=wvt_sb, in_=chunked(wvt_d, ROWS))
            nc.scalar.dma_start(
                out=xb_sb[:, :, 2 * TQ_U : T], in_=chunked(xb_d, T, 2 * TQ_U, T)
            )

            # q/k per pair: (128, T) bf16; head pair p covers heads 2p, 2p+1
            q_sb = [
                singles.tile([128, T], BF16, tag=f"q{p}", name=f"q{p}")
                for p in range(N_PAIRS)
            ]
            k_sb = [
                singles.tile([128, T], BF16, tag=f"k{p}", name=f"k{p}")
                for p in range(N_PAIRS)
            ]
            # per tk-tile, per head: [vT | ones] (65 columns, ones last)
            vt_aug = singles.tile([128, TK_TILES, H_LOC, DK + 1], BF16, tag="vt")

            def emit_proj_piece(p, u, wt_sb, dst):
                """One 512-col piece (pair p, quarter u) of a q/k projection."""
                ps = psS.tile([128, TQ_U], FP32, tag="S", name="projps")
                col = TQ_U * u
                for c in range(2):
                    nc.tensor.matmul(
                        ps,
                        lhsT=wt_sb[:, c, 128 * p : 128 * (p + 1)],
                        rhs=xb_sb[:, c, col : col + TQ_U],
                        start=(c == 0),
                        stop=(c == 1),
                    )
                nc.vector.tensor_copy(dst[:, col : col + TQ_U], ps)

            def emit_vt_pair(i):
                """vT for tk-tiles i, i+1 computed into one pool tile."""
                ps = psS.tile([128, 2, H_LOC, DK], FP32, tag="S", name="vtps")
                for t in range(2):
                    for c in range(2):
                        nc.tensor.matmul(
                            ps[:, t],
                            lhsT=xb_sb[:, c, 128 * (i + t) : 128 * (i + t + 1)],
                            rhs=wvt_sb[:, c],
                            start=(c == 0),
                            stop=(c == 1),
                        )
                nc.vector.tensor_copy(vt_aug[:, i : i + 2, :, 0:DK], ps)

            def emit_unit(p, u, interleave=(), defer_tail=True,
                          greedy_items=False, u_idx=0):
                """One (pair, quarter) unit: 16 steps in 4 groups of 4.
                Scores+exp run solid within a group (pure 64x128 PE mode);
                at each group boundary the previous group's attn@V matmuls
                flush as one 128x128-mode block, then up to two `interleave`
                items run. The last group's flush + epilogue are returned as
                closures for the next unit (defer_tail)."""
                interleave = list(interleave)
                # transposed attn@V: acc[h] is [tq_part, chunk, d|sumexp];
                # lhsT = E chunk (tk, tq128), rhs = vt (tk, d|ones): N=65
                # streamed -> ~4x cheaper per element than the (65, 512)
                # orientation, and sumexp lands PER PARTITION (col 64).
                # chunk stride padded to 128 f32 (512B) so each region's
                # start=True zeroing cannot clobber sibling chunks in the bank
                accs = [
                    psA.tile([128, 4, 128], FP32, tag="acc", name=f"acc{h}")
                    for h in range(2)
                ]
                e_tiles = [None] * TK_TILES

                def emit_mm2(j):
                    for h in range(2):
                        for c in range(4):
                            nc.tensor.matmul(
                                accs[h][:, c, 0 : DK + 1],
                                lhsT=e_tiles[j][
                                    :,
                                    TQ_U * h + 128 * c : TQ_U * h + 128 * (c + 1),
                                ],
                                rhs=vt_aug[:, j, 2 * p + h, :],
                                start=(j == 0 and c == 0),
                                stop=(j == TK_TILES - 1 and c == 3),
                            )

                def pop_items(n):
                    while n > 0 and interleave:
                        nxt = interleave.pop(0)
                        n -= 1
                        if nxt is not None:
                            nxt()

                for j in range(TK_TILES):
                  with tc.tile_wait_until(base + pace * (TK_TILES * u_idx + j),
                                          enable=pace > 0):
                    s_tile = psS.tile([128, 2 * TQ_U], FP32, tag="S", name="s")
                    for h in range(2):
                        hb = 0 if sc_base0 else h
                        nc.tensor.matmul(
                            s_tile[:, TQ_U * h : TQ_U * (h + 1)],
                            lhsT=k_sb[p][
                                64 * hb : 64 * hb + 64, 128 * j : 128 * (j + 1)
                            ],
                            rhs=q_sb[p][
                                64 * hb : 64 * hb + 64, TQ_U * u : TQ_U * (u + 1)
                            ],
                            start=True,
                            stop=True,
                        )
                    e = eP.tile([128, 2 * TQ_U], BF16, tag="E", name="e")
                    if j in dve_steps:
                        nc.vector._custom_dve(
                            EXPOP, out=e, in0=s_tile,
                            s0=EXP_C0, s1=EXP_C1, imm2=EXP_C2,
                        )
                    else:
                        nc.scalar.activation(e, s_tile, EXP, scale=0.125)
                    e_tiles[j] = e
                    if j >= lag:
                        emit_mm2(j - lag)
                    pop_items(1)

                def emit_epilogue():
                    for h in range(2):
                        hl = 2 * p + h
                        # evacuate transposed acc (1 bank) -> SBUF
                        av = outP.tile([128, 4, DK + 1], FP32, tag="av", name="av")
                        nc.vector.tensor_copy(av, accs[h][:, :, 0 : DK + 1])
                        # per-partition softmax divide on GPSIMD, bf16 out
                        ot = outP.tile([128, 4, DK], BF16, tag="ot", name="ot")
                        for c in range(4):
                            nc.gpsimd.normalize_recip(
                                ot[:, c, :],
                                av[:, c, 0:DK],
                                av[:, c, DK : DK + 1],
                            )
                        # xbar transpose (tq128, 2x64) -> ((c,d)128, tq128)
                        for b in range(2):
                            tr = small.tile([128, 128], BF16, tag="tr", name="tr")
                            nc.sync.dma_start_transpose(
                                out=tr, in_=ot[:, 2 * b : 2 * b + 2, :]
                            )
                            for cc in range(2):
                                col = TQ_U * u + 128 * (2 * b + cc)
                                nc.sync.dma_start(
                                    out=out_d[
                                        DK * hl : DK * (hl + 1), col : col + 128
                                    ],
                                    in_=tr[64 * cc : 64 * cc + 64, :],
                                )

                while interleave:
                    nxt = interleave.pop(0)
                    if nxt is not None:
                        nxt()

                def tail_a():
                    for j in range(TK_TILES - lag, TK_TILES):
                        emit_mm2(j)

                def tail_b():
                    emit_epilogue()

                if defer_tail:
                    return [tail_a, tail_b]
                tail_a()
                tail_b()
                return []

            # ---- emission order ----
            from functools import partial

            def qp(p, u):
                return partial(emit_proj_piece, p, u, wqt_sb, q_sb[p])

            def kp(p, u):
                return partial(emit_proj_piece, p, u, wkt_sb, k_sb[p])

            def spread(items, lead=1):
                out = [None] * lead
                for it in items:
                    out.extend([it, None])
                return out

            def emit_pass():
                nc.gpsimd.memset(vt_aug, 1.0)
                # unblock the first unit: k pair0 (all quarters), q pair0 q0
                for u in range(N_Q):
                    emit_proj_piece(0, u, wkt_sb, k_sb[0])
                emit_proj_piece(0, 0, wqt_sb, q_sb[0])
                vt = [partial(emit_vt_pair, 2 * i) for i in range(8)]
                # Item slots are popped one per step AFTER that step's exp is
                # emitted. DVE-queue discipline: the deferred epilogue enters
                # at steps 0-1 (prev tail); proj/vT copies go in the DVE idle
                # windows BETWEEN its exp steps (5, 8, 11, 14), never before
                # one, so DVE exps stay early and the score-slot rotation
                # (scores j+1 gate on exp j-2) never stalls ScalarE.
                il = {
                    (0, 0): [vt[0], vt[1], vt[2], vt[3], vt[4], None,
                             vt[5], qp(0, 1), None, vt[6], qp(0, 2), None,
                             vt[7], qp(0, 3)],
                    # prev_tail occupies steps 0-1; il[i] lands at step 2+i
                    (0, 1): [None, None, None, None, kp(1, 0), None, None,
                             kp(1, 1), None, None, kp(1, 2), None, None,
                             kp(1, 3)],
                    (0, 2): [None, None, None, None, qp(1, 0), None, None,
                             qp(1, 1), None, None, qp(1, 2), None, None,
                             qp(1, 3)],
                    (0, 3): [],
                    (1, 0): [],
                    (1, 1): [],
                    (1, 2): [],
                    (1, 3): [],
                }
                prev_tail = []
                order = [(p, u) for p in range(N_PAIRS) for u in range(N_Q)]
                for idx, (p, u) in enumerate(order):
                    items = prev_tail + list(il[(p, u)])
                    last = idx == len(order) - 1
                    prev_tail = emit_unit(
                        p, u, items, defer_tail=not last,
                        greedy_items=(idx == 0), u_idx=idx,
                    )

            if loop_n is not None:
                with tc.For_i(0, loop_n, 1):
                    emit_pass()
            else:
                for _ in range(passes):
                    emit_pass()

    nc.compile()
    return nc


def _get_program(passes=1, loop_n=None, **kw):
    key = (passes, loop_n, tuple(sorted(kw.items())))
    if key not in _PROGRAMS:
        _PROGRAMS[key] = _build_program(passes, loop_n, **kw)
    return _PROGRAMS[key]


def _make_in_maps(inputs):
    x = np.asarray(inputs["x"])
    Wq = np.asarray(inputs["Wq"])
    Wk = np.asarray(inputs["Wk"])
    Wv = np.asarray(inputs["Wv"])
    xb = [np.ascontiguousarray(x[n]).astype(ml_dtypes.bfloat16) for n in range(N_BATCH)]
    rows = [slice(ROWS * g, ROWS * (g + 1)) for g in range(2)]

    def wt(w, r):
        return np.ascontiguousarray(w[r].T).astype(ml_dtypes.bfloat16)

    wqt = [wt(Wq, r) for r in rows]
    wkt = [wt(Wk, r) for r in rows]
    wvt = [wt(Wv, r) for r in rows]
    return [
        {"xb": xb[c // 2], "wqt": wqt[c % 2], "wkt": wkt[c % 2], "wvt": wvt[c % 2]}
        for c in range(N_CORES)
    ]


_CALLABLE = None


def _get_callable():
    """Build the sharded PJRT callable once; repeated kernel() calls reuse it."""
    global _CALLABLE
    if _CALLABLE is not None:
        return _CALLABLE
    import jax
    from jax.sharding import Mesh, PartitionSpec

    from jax.experimental.shard_map import shard_map
    import concourse.bass2jax as b2j
    from concourse import mybir

    nc = _get_program()
    b2j.install_neuronx_cc_hook()
    partition_name = nc.partition_id_tensor.name if nc.partition_id_tensor else None
    in_names, out_names, out_avals, zero_outs = [], [], [], []
    for alloc in nc.m.functions[0].allocations:
        if not isinstance(alloc, mybir.MemoryLocationSet):
            continue
        name = alloc.memorylocations[0].name
        if alloc.kind == "ExternalInput":
            if name != partition_name:
                in_names.append(name)
        elif alloc.kind == "ExternalOutput":
            shape = tuple(alloc.tensor_shape)
            dtype = mybir.dt.np(alloc.dtype)
            out_names.append(name)
            out_avals.append(jax.core.ShapedArray(shape, dtype))
            zero_outs.append(np.zeros(shape, dtype))
    n_params = len(in_names)
    all_in_names = list(in_names) + list(out_names)
    if partition_name is not None:
        all_in_names.append(partition_name)

    def _body(*args):
        operands = list(args)
        if partition_name is not None:
            operands.append(b2j.partition_id_tensor())
        outs = b2j._bass_exec_p.bind(
            *operands,
            out_avals=tuple(out_avals),
            in_names=tuple(all_in_names),
            out_names=tuple(out_names),
            lowering_input_output_aliases=(),
            sim_require_finite=True,
            sim_require_nnan=True,
            nc=nc,
        )
        return tuple(outs)

    devices = jax.devices()[:N_CORES]
    mesh = Mesh(np.asarray(devices), ("core",))
    in_specs = (PartitionSpec("core"),) * (n_params + len(out_names))
    out_specs = (PartitionSpec("core"),) * len(out_names)
    fn = jax.jit(
        shard_map(
            _body, mesh=mesh, in_specs=in_specs, out_specs=out_specs,
            check_rep=False,
        ),
        keep_unused=True,
    )
    concat_zeros = [
        np.zeros((N_CORES * z.shape[0], *z.shape[1:]), z.dtype) for z in zero_outs
    ]
    _CALLABLE = (fn, in_names, out_names, out_avals, concat_zeros)
    return _CALLABLE


def kernel(x, Wq, Wk, Wv):
    fn, in_names, out_names, out_avals, concat_zeros = _get_callable()
    in_maps = _make_in_maps({"x": x, "Wq": Wq, "Wk": Wk, "Wv": Wv})
    concat_in = [
        np.concatenate([in_maps[c][nm] for c in range(N_CORES)], axis=0)
        for nm in in_names
    ]
    out_arrs = fn(*concat_in, *concat_zeros)
    oi = out_names.index("out")
    res = np.asarray(out_arrs[oi]).reshape(N_CORES, *out_avals[oi].shape)

    out = np.empty((N_BATCH, C_OUT, T), np.float32)
    for c in range(N_CORES):
        n = c // 2
        g = c % 2
        out[n, ROWS * g : ROWS * (g + 1), :] = res[c]
    return out


if __name__ == "__main__":
    xs = np.random.randn(N_BATCH, C_IN, T).astype(np.float32)
    wq = (np.random.randn(C_OUT, C_IN) * 0.02).astype(np.float32)
    wk = (np.random.randn(C_OUT, C_IN) * 0.02).astype(np.float32)
    wv = (np.random.randn(C_OUT, C_IN) * 0.02).astype(np.float32)
    o = kernel(xs, wq, wk, wv)
    print("out", o.shape, o.dtype, np.abs(o).max())


# revision 29
# speedup vs baseline: 1.1620x; 1.0980x over previous
"""MultiHeadAttn1D Trainium2 Bass kernel (dual-engine exp + PE row tiling).

Problem: x (4, 256, 2048) fp32; Wq/Wk (512, 256); Wv (512, 256).
  q = Wq @ x[n]; k = Wk @ x[n]; v = Wv @ x[n]  (per batch n)
  per head h (8 heads, dk=dv=64):
    scores[tk, tq] = sum_d k[d,tk] q[d,tq] / 8; attn = softmax over tk
    out[d, tq] = sum_tk attn[tk,tq] v[d,tk]

Sharding: 8 cores = 4 batch x 2 head-groups. Core c handles n = c//2 and
heads 4*(c%2) .. 4*(c%2)+3 (256 rows of each W). Pure SPMD, no collectives.

Per-core design (matmuls bf16 operands, fp32 PSUM accumulate):
  - Head-PAIR layout: the 4 local heads form 2 pairs; a pair's q/k rows sit
    at partitions 0-63 (even head) / 64-127 (odd head) of one (128, T)
    projection tile. The two K=64 score matmuls of a step (one per head)
    land on disjoint 64-row PE row-groups (tile_position auto-derives from
    the base partitions) and different PSUM banks, so they execute
    CONCURRENTLY in the PE array - measured ~84ns/matmul in a clean stream
    vs 300ns serial; disabling the row split costs +30us on the full
    kernel. No weight duplication needed (v1 doubled q/k projection work).
  - exp runs on TWO engines. ScalarE activation takes 12 of 16 tk-tiles
    per unit at (172+FD)/1.2 ns from PSUM (997ns @ FD=1024); a custom DVE
    op EXP16_ANT (registered additively into dve_ops at import) takes
    steps {5,8,11,14}, reading PSUM directly at 1 elem/cycle/lane. The op
    computes ((c0*s+c1)^2+c2)^16 ~= exp(s/8) via 4 inline squarings
    (8-stage ALU chain; scores satisfy |s| < ~5, fit window |s|<=12; rel
    err ~7e-4 fp32, bf16 cast dominates). A pure mul/add chain is safe
    from PSUM - the bitwise-NOT reciprocal op is the one that misbehaves.
    Both engines write bf16 E tiles; softmax mixes them freely per row.
  - Unit = (head-pair, tq-quarter of 512): 16 steps, one tk-tile each:
    2 concurrent score matmuls -> one FD=1024 exp (both heads) -> TRANSPOSED
    attn@V, lagged 3 steps: lhsT = E chunk (tk, tq128), rhs = vt (tk,
    d|ones), so each matmul streams only N=65 (~4x cheaper than the
    (65, 512) orientation) and the accumulator is [tq_part, chunk, d|sum]
    with sum(exp) landing PER PARTITION in column 64. PSUM pending-zero is
    BANK-granular: exactly one start=True per acc bank (j=0, c=0) - a
    start per chunk re-marks the bank and drops sibling j=0 contributions.
    fp8 attn@V (DoubleRow) was measured 2.3x faster still but fails the
    accuracy budget (~5-7% output error from e4m3 E/v quantization).
  - PSUM: 3 score slots (128,1024)f32 = 6 banks + 2 transposed accs
    (128,4,128)f32 = 1 bank each = 8 exactly.
  - Epilogue per unit per head: one small evac copy (DVE, 396ns), then
    GPSIMD normalize_recip divides by the per-partition sumexp and emits
    bf16, xbar dma_start_transpose flips (tq,128x(c,d)) blocks to output
    orientation, and plain DMAs store bf16 to DRAM (kernel() upcasts to
    fp32 on host; bf16 out costs ~3e-4 extra error). This removes the
    sum-row copy, reciprocal, broadcast, and multiply from the DVE/GPSIMD
    critical path entirely. The last `lag` attn@V steps + the epilogue are
    deferred into the next unit's first interleave slots.
  - Projection pieces / vT pairs / deferred tails interleave one per step
    into the score stream. Scheduler pacing via tile_wait_until was tried
    (pace knob) and measured neutral-to-negative; disabled by default.
"""

import numpy as np
import ml_dtypes

# Problem constants (hardcoded per contract; kernel.py must be self-contained)
N_BATCH = 4
C_IN = 256
T = 2048
C_OUT = 512
H = 8
DK = 64
N_CORES = 8
H_LOC = 4            # heads per core
ROWS = 256           # W rows per core (H_LOC * DK)
TK_TILES = 16        # T / 128
TQ_U = 512           # tq per unit (quarter of T)
N_PAIRS = 2          # head pairs per core
N_Q = 4              # tq quarters

# exp(s/8) ~= ((C0*s + C1)^2 + C2)^16, fit on |s|<=12 (see docstring)
EXP_C0 = 0.005531007068092956
EXP_C1 = 0.7070977101176488
EXP_C2 = 0.5000073251988786

# DVE takes these tk-tile exps in each unit (rest on ScalarE). Placement
# rule (worth ~17us): no two steps may differ by 3 = the score-slot count,
# else scores j+3 gates on DVE exp j and the DVE exps chain serially into
# the score critical path, starving ScalarE ((5,8,11,14) measured 131.7us,
# (3,6,9,12,15) 154us, this set 114.4us). Step 15 on DVE also loses (the
# deferred tail attn@V waits on it at the next unit's start): x=6 with
# (2,4,9,11,13,15) measured 139.6us.
DVE_STEPS = (0, 2, 4, 9, 11, 13)

_PROGRAMS = {}


def _register_exp_op():
    """Register EXP16_ANT as a custom DVE op (idempotent, additive)."""
    from concourse import dve_ops
    from concourse.dve_spec import C0, C1, C2, Spec, Src0, lower
    from concourse.dve_table_gen import dve_ver_for
    from concourse.dve_uop import DveOpSpec

    name = "EXP16_ANT"
    for op in dve_ops.OPS:
        if op.name == name:
            return op

    def ref(in0, in1, s0, s1, imm2):
        u = (np.float32(s0) * in0.astype(np.float32) + np.float32(s1)).astype(
            np.float32
        )
        q = (u * u + np.float32(imm2)).astype(np.float32)
        q2 = (q * q).astype(np.float32)
        q4 = (q2 * q2).astype(np.float32)
        q8 = (q4 * q4).astype(np.float32)
        return (q8 * q8).astype(np.float32)

    u = Src0 * C0 + C1
    q = u * u + C2
    q2 = q * q
    q4 = q2 * q2
    q8 = q4 * q4
    spec = Spec(body=q8 * q8, reference=ref)

    ver = dve_ver_for("TRN2")
    row = dve_ops._CUSTOM_DVE_ROW_BASE + len(dve_ops.OPS)
    tmp = DveOpSpec(name=name, opcode=row, uops=lower(spec, ver=ver), rd1_en=False)
    op = dve_ops.DveOp(name, spec, subdim=False, uops_sha={ver: tmp.sha(ver)})
    dve_ops.OPS.append(op)
    dve_ops.CUSTOM_DVE_SPECS[name] = spec
    dve_ops._SUB_OPCODE_FOR_NAME[name] = row
    return op


def _build_program(passes=1, loop_n=None, dve_steps=DVE_STEPS, lag=3,
                   mult_gpsimd=True, e_bufs=12, pace=0, base=0.004,
                   sc_base0=False):
    import concourse.bass as bass  # noqa: F401
    import concourse.tile as tile
    from concourse import bacc, mybir

    BF16 = mybir.dt.bfloat16
    FP32 = mybir.dt.float32
    EXP = mybir.ActivationFunctionType.Exp
    EXPOP = _register_exp_op()

    nc = bacc.Bacc(
        "TRN2",
        target_bir_lowering=False,
        debug=False,
        num_devices=N_CORES,
    )

    xb_d = nc.dram_tensor("xb", [C_IN, T], BF16, kind="ExternalInput").ap()
    wqt_d = nc.dram_tensor("wqt", [C_IN, ROWS], BF16, kind="ExternalInput").ap()
    wkt_d = nc.dram_tensor("wkt", [C_IN, ROWS], BF16, kind="ExternalInput").ap()
    wvt_d = nc.dram_tensor("wvt", [C_IN, ROWS], BF16, kind="ExternalInput").ap()
    out_d = nc.dram_tensor("out", [ROWS, T], BF16, kind="ExternalOutput").ap()

    with tile.TileContext(nc) as tc:
        from contextlib import ExitStack

        with ExitStack() as ctx:
            singles = ctx.enter_context(tc.tile_pool(name="singles", bufs=1))
            psS = ctx.enter_context(tc.tile_pool(name="psS", bufs=3, space="PSUM"))
            psA = ctx.enter_context(tc.tile_pool(name="psA", bufs=2, space="PSUM"))
            eP = ctx.enter_context(tc.tile_pool(name="eP", bufs=e_bufs))
            small = ctx.enter_context(tc.tile_pool(name="small", bufs=6))
            outP = ctx.enter_context(tc.tile_pool(name="outP", bufs=6))

            # ---- persistent SBUF tensors ----
            xb_sb = singles.tile([128, 2, T], BF16, tag="xb", name="xb_sb")
            wqt_sb = singles.tile([128, 2, ROWS], BF16, tag="wqt", name="wqt_sb")
            wkt_sb = singles.tile([128, 2, ROWS], BF16, tag="wkt", name="wkt_sb")
            wvt_sb = singles.tile([128, 2, ROWS], BF16, tag="wvt", name="wvt_sb")

            def chunked(dram_ap, cols, c0=0, c1=None):
                """(256, F) dram AP -> (128, 2, c1-c0) view, chunk-major free."""
                c1 = cols if c1 is None else c1
                import concourse.bass as bass_mod

                return bass_mod.AP(
                    tensor=dram_ap.tensor,
                    offset=dram_ap.offset + c0,
                    ap=[[cols, 128], [128 * cols, 2], [1, c1 - c0]],
                )

            nc.sync.dma_start(out=wkt_sb, in_=chunked(wkt_d, ROWS))
            nc.scalar.dma_start(
                out=xb_sb[:, :, 0:TQ_U], in_=chunked(xb_d, T, 0, TQ_U)
            )
            nc.sync.dma_start(out=wqt_sb, in_=chunked(wqt_d, ROWS))
            nc.scalar.dma_start(
                out=xb_sb[:, :, TQ_U : 2 * TQ_U], in_=chunked(xb_d, T, TQ_U, 2 * TQ_U)
            )
            nc.sync.dma_start(out=wvt_sb, in_=chunked(wvt_d, ROWS))
            nc.scalar.dma_start(
                out=xb_sb[:, :, 2 * TQ_U : T], in_=chunked(xb_d, T, 2 * TQ_U, T)
            )

            # q/k per pair: (128, T) bf16; head pair p covers heads 2p, 2p+1
            q_sb = [
                singles.tile([128, T], BF16, tag=f"q{p}", name=f"q{p}")
                for p in range(N_PAIRS)
            ]
            k_sb = [
                singles.tile([128, T], BF16, tag=f"k{p}", name=f"k{p}")
                for p in range(N_PAIRS)
            ]
            # per tk-tile, per head: [vT | ones] (65 columns, ones last)
            vt_aug = singles.tile([128, TK_TILES, H_LOC, DK + 1], BF16, tag="vt")

            def emit_proj_piece(p, u, wt_sb, dst):
                """One 512-col piece (pair p, quarter u) of a q/k projection."""
                ps = psS.tile([128, TQ_U], FP32, tag="S", name="projps")
                col = TQ_U * u
                for c in range(2):
                    nc.tensor.matmul(
                        ps,
                        lhsT=wt_sb[:, c, 128 * p : 128 * (p + 1)],
                        rhs=xb_sb[:, c, col : col + TQ_U],
                        start=(c == 0),
                        stop=(c == 1),
                    )
                nc.vector.tensor_copy(dst[:, col : col + TQ_U], ps)

            def emit_vt_pair(i):
                """vT for tk-tiles i, i+1 computed into one pool tile."""
                ps = psS.tile([128, 2, H_LOC, DK], FP32, tag="S", name="vtps")
                for t in range(2):
                    for c in range(2):
                        nc.tensor.matmul(
                            ps[:, t],
                            lhsT=xb_sb[:, c, 128 * (i + t) : 128 * (i + t + 1)],
                            rhs=wvt_sb[:, c],
                            start=(c == 0),
                            stop=(c == 1),
                        )
                nc.vector.tensor_copy(vt_aug[:, i : i + 2, :, 0:DK], ps)

            def emit_unit(p, u, interleave=(), defer_tail=True,
                          greedy_items=False, u_idx=0):
                """One (pair, quarter) unit: 16 steps in 4 groups of 4.
                Scores+exp run solid within a group (pure 64x128 PE mode);
                at each group boundary the previous group's attn@V matmuls
                flush as one 128x128-mode block, then up to two `interleave`
                items run. The last group's flush + epilogue are returned as
                closures for the next unit (defer_tail)."""
                interleave = list(interleave)
                # transposed attn@V: acc[h] is [tq_part, chunk, d|sumexp];
                # lhsT = E chunk (tk, tq128), rhs = vt (tk, d|ones): N=65
                # streamed -> ~4x cheaper per element than the (65, 512)
                # orientation, and sumexp lands PER PARTITION (col 64).
                # chunk stride padded to 128 f32 (512B) so each region's
                # start=True zeroing cannot clobber sibling chunks in the bank
                accs = [
                    psA.tile([128, 4, 128], FP32, tag="acc", name=f"acc{h}")
                    for h in range(2)
                ]
                e_tiles = [None] * TK_TILES

                def emit_mm2(j):
                    for h in range(2):
                        for c in range(4):
                            nc.tensor.matmul(
                                accs[h][:, c, 0 : DK + 1],
                                lhsT=e_tiles[j][
                                    :,
                                    TQ_U * h + 128 * c : TQ_U * h + 128 * (c + 1),
                                ],
                                rhs=vt_aug[:, j, 2 * p + h, :],
                                start=(j == 0 and c == 0),
                                stop=(j == TK_TILES - 1 and c == 3),
                            )

                def pop_items(n):
                    while n > 0 and interleave:
                        nxt = interleave.pop(0)
                        n -= 1
                        if nxt is not None:
                            nxt()

                for j in range(TK_TILES):
                  with tc.tile_wait_until(base + pace * (TK_TILES * u_idx + j),
                                          enable=pace > 0):
                    s_tile = psS.tile([128, 2 * TQ_U], FP32, tag="S", name="s")
                    for h in range(2):
                        hb = 0 if sc_base0 else h
                        nc.tensor.matmul(
                            s_tile[:, TQ_U * h : TQ_U * (h + 1)],
                            lhsT=k_sb[p][
                                64 * hb : 64 * hb + 64, 128 * j : 128 * (j + 1)
                            ],
                            rhs=q_sb[p][
                                64 * hb : 64 * hb + 64, TQ_U * u : TQ_U * (u + 1)
                            ],
                            start=True,
                            stop=True,
                        )
                    e = eP.tile([128, 2 * TQ_U], BF16, tag="E", name="e")
                    if j in dve_steps:
                        nc.vector._custom_dve(
                            EXPOP, out=e, in0=s_tile,
                            s0=EXP_C0, s1=EXP_C1, imm2=EXP_C2,
                        )
                    else:
                        nc.scalar.activation(e, s_tile, EXP, scale=0.125)
                    e_tiles[j] = e
                    if j >= lag:
                        emit_mm2(j - lag)
                    pop_items(1)

                def emit_epilogue():
                    for h in range(2):
                        hl = 2 * p + h
                        # evacuate transposed acc (1 bank) -> SBUF
                        av = outP.tile([128, 4, DK + 1], FP32, tag="av", name="av")
                        nc.vector.tensor_copy(av, accs[h][:, :, 0 : DK + 1])
                        # per-partition softmax divide on GPSIMD, bf16 out
                        ot = outP.tile([128, 4, DK], BF16, tag="ot", name="ot")
                        for c in range(4):
                            nc.gpsimd.normalize_recip(
                                ot[:, c, :],
                                av[:, c, 0:DK],
                                av[:, c, DK : DK + 1],
                            )
                        # xbar transpose (tq128, 2x64) -> ((c,d)128, tq128)
                        for b in range(2):
                            tr = small.tile([128, 128], BF16, tag="tr", name="tr")
                            nc.sync.dma_start_transpose(
                                out=tr, in_=ot[:, 2 * b : 2 * b + 2, :]
                            )
                            for cc in range(2):
                                col = TQ_U * u + 128 * (2 * b + cc)
                                nc.sync.dma_start(
                                    out=out_d[
                                        DK * hl : DK * (hl + 1), col : col + 128
                                    ],
                                    in_=tr[64 * cc : 64 * cc + 64, :],
                                )

                while interleave:
                    nxt = interleave.pop(0)
                    if nxt is not None:
                        nxt()

                def tail_a():
                    for j in range(TK_TILES - lag, TK_TILES):
                        emit_mm2(j)

                def tail_b():
                    emit_epilogue()

                if defer_tail:
                    return [tail_a, tail_b]
                tail_a()
                tail_b()
                return []

            # ---- emission order ----
            from functools import partial

            def qp(p, u):
                return partial(emit_proj_piece, p, u, wqt_sb, q_sb[p])

            def kp(p, u):
                return partial(emit_proj_piece, p, u, wkt_sb, k_sb[p])

            def spread(items, lead=1):
                out = [None] * lead
                for it in items:
                    out.extend([it, None])
                return out

            def emit_pass():
                nc.gpsimd.memset(vt_aug, 1.0)
                # unblock the first unit: k pair0 (all quarters), q pair0 q0
                for u in range(N_Q):
                    emit_proj_piece(0, u, wkt_sb, k_sb[0])
                emit_proj_piece(0, 0, wqt_sb, q_sb[0])
                vt = [partial(emit_vt_pair, 2 * i) for i in range(8)]
                # Item slots are popped one per step AFTER that step's exp is
                # emitted. DVE-queue discipline: the deferred epilogue enters
                # at steps 0-1 (prev tail); proj/vT copies go in the DVE idle
                # windows BETWEEN its exp steps (5, 8, 11, 14), never before
                # one, so DVE exps stay early and the score-slot rotation
                # (scores j+1 gate on exp j-2) never stalls ScalarE.
                il = {
                    (0, 0): [vt[0], vt[1], vt[2], vt[3], vt[4], None,
                             vt[5], qp(0, 1), None, vt[6], qp(0, 2), None,
                             vt[7], qp(0, 3)],
                    # prev_tail occupies steps 0-1; il[i] lands at step 2+i
                    (0, 1): [None, None, None, None, kp(1, 0), None, None,
                             kp(1, 1), None, None, kp(1, 2), None, None,
                             kp(1, 3)],
                    (0, 2): [None, None, None, None, qp(1, 0), None, None,
                             qp(1, 1), None, None, qp(1, 2), None, None,
                             qp(1, 3)],
                    (0, 3): [],
                    (1, 0): [],
                    (1, 1): [],
                    (1, 2): [],
                    (1, 3): [],
                }
                prev_tail = []
                order = [(p, u) for p in range(N_PAIRS) for u in range(N_Q)]
                for idx, (p, u) in enumerate(order):
                    items = prev_tail + list(il[(p, u)])
                    last = idx == len(order) - 1
                    prev_tail = emit_unit(
                        p, u, items, defer_tail=not last,
                        greedy_items=(idx == 0), u_idx=idx,
                    )

            if loop_n is not None:
                with tc.For_i(0, loop_n, 1):
                    emit_pass()
            else:
                for _ in range(passes):
                    emit_pass()

    nc.compile()
    return nc


def _get_program(passes=1, loop_n=None, **kw):
    key = (passes, loop_n, tuple(sorted(kw.items())))
    if key not in _PROGRAMS:
        _PROGRAMS[key] = _build_program(passes, loop_n, **kw)
    return _PROGRAMS[key]


def _make_in_maps(inputs):
    x = np.asarray(inputs["x"])
    Wq = np.asarray(inputs["Wq"])
    Wk = np.asarray(inputs["Wk"])
    Wv = np.asarray(inputs["Wv"])
    xb = [np.ascontiguousarray(x[n]).astype(ml_dtypes.bfloat16) for n in range(N_BATCH)]
    rows = [slice(ROWS * g, ROWS * (g + 1)) for g in range(2)]

    def wt(w, r):
        return np.ascontiguousarray(w[r].T).astype(ml_dtypes.bfloat16)

    wqt = [wt(Wq, r) for r in rows]
    wkt = [wt(Wk, r) for r in rows]
    wvt = [wt(Wv, r) for r in rows]
    return [
        {"xb": xb[c // 2], "wqt": wqt[c % 2], "wkt": wkt[c % 2], "wvt": wvt[c % 2]}
        for c in range(N_CORES)
    ]


_CALLABLE = None


def _get_callable():
    """Build the sharded PJRT callable once; repeated kernel() calls reuse it."""
    global _CALLABLE
    if _CALLABLE is not None:
        return _CALLABLE
    import jax
    from jax.sharding import Mesh, PartitionSpec

    from jax.experimental.shard_map import shard_map
    import concourse.bass2jax as b2j
    from concourse import mybir

    nc = _get_program()
    b2j.install_neuronx_cc_hook()
    partition_name = nc.partition_id_tensor.name if nc.partition_id_tensor else None
    in_names, out_names, out_avals, zero_outs = [], [], [], []
    for alloc in nc.m.functions[0].allocations:
        if not isinstance(alloc, mybir.MemoryLocationSet):
            continue
        name = alloc.memorylocations[0].name
        if alloc.kind == "ExternalInput":
            if name != partition_name:
                in_names.append(name)
        elif alloc.kind == "ExternalOutput":
            shape = tuple(alloc.tensor_shape)
            dtype = mybir.dt.np(alloc.dtype)
            out_names.append(name)
            out_avals.append(jax.core.ShapedArray(shape, dtype))
            zero_outs.append(np.zeros(shape, dtype))
    n_params = len(in_names)
    all_in_names = list(in_names) + list(out_names)
    if partition_name is not None:
        all_in_names.append(partition_name)

    def _body(*args):
        operands = list(args)
        if partition_name is not None:
            operands.append(b2j.partition_id_tensor())
        outs = b2j._bass_exec_p.bind(
            *operands,
            out_avals=tuple(out_avals),
            in_names=tuple(all_in_names),
            out_names=tuple(out_names),
            lowering_input_output_aliases=(),
            sim_require_finite=True,
            sim_require_nnan=True,
            nc=nc,
        )
        return tuple(outs)

    devices = jax.devices()[:N_CORES]
    mesh = Mesh(np.asarray(devices), ("core",))
    in_specs = (PartitionSpec("core"),) * (n_params + len(out_names))
    out_specs = (PartitionSpec("core"),) * len(out_names)
    fn = jax.jit(
        shard_map(
            _body, mesh=mesh, in_specs=in_specs, out_specs=out_specs,
            check_rep=False,
        ),
        keep_unused=True,
    )
    concat_zeros = [
        np.zeros((N_CORES * z.shape[0], *z.shape[1:]), z.dtype) for z in zero_outs
    ]
    _CALLABLE = (fn, in_names, out_names, out_avals, concat_zeros)
    return _CALLABLE


def kernel(x, Wq, Wk, Wv):
    fn, in_names, out_names, out_avals, concat_zeros = _get_callable()
    in_maps = _make_in_maps({"x": x, "Wq": Wq, "Wk": Wk, "Wv": Wv})
    concat_in = [
        np.concatenate([in_maps[c][nm] for c in range(N_CORES)], axis=0)
        for nm in in_names
    ]
    out_arrs = fn(*concat_in, *concat_zeros)
    oi = out_names.index("out")
    res = np.asarray(out_arrs[oi]).reshape(N_CORES, *out_avals[oi].shape)

    out = np.empty((N_BATCH, C_OUT, T), np.float32)
    for c in range(N_CORES):
        n = c // 2
        g = c % 2
        out[n, ROWS * g : ROWS * (g + 1), :] = res[c]
    return out


if __name__ == "__main__":
    xs = np.random.randn(N_BATCH, C_IN, T).astype(np.float32)
    wq = (np.random.randn(C_OUT, C_IN) * 0.02).astype(np.float32)
    wk = (np.random.randn(C_OUT, C_IN) * 0.02).astype(np.float32)
    wv = (np.random.randn(C_OUT, C_IN) * 0.02).astype(np.float32)
    o = kernel(xs, wq, wk, wv)
    print("out", o.shape, o.dtype, np.abs(o).max())
